# revision 3
# baseline (speedup 1.0000x reference)
"""Trainium2 Bass kernel for nn_CapsuleNetwork (self-contained).

Math (reference, with IN_CAPS == 1):
  u = x @ Wf.T                      # [B, 1024], Wf = W.reshape(1024, 1024)
  usq[b,n] = sum_d u[b, n*256+d]^2  # capsule norms
  routing (2 iters) collapses to scalar math on usq -> alpha[b,n]
  v = alpha * u (per capsule)       # [B, 4, 256]
  batchnorm over (B, n) per channel dc -> fold into W1
  out = relu(v_bn @ W1.T + b1) @ W2.T + b2

Distribution: data-parallel over batch across 8 cores; BN stats all-reduced.

Precision: layer-1 matmul in float32r (hw ~1.4e-4), routing math fp32,
v stored bf16, MLP in bf16, output fp32.
"""
import numpy as np
import ml_dtypes

import concourse.bass as bass
import concourse.mybir as mybir
import concourse.tile as tile
from concourse import bacc
from concourse.bass_utils import run_bass_kernel_spmd
from concourse.masks import make_identity

dt = mybir.dt
AF = mybir.ActivationFunctionType
ALU = mybir.AluOpType

NCORES = 8
B_TOTAL = 32768
DIM = 1024
NCAPS = 4
DCAPS = 256
BN_EPS = 1e-5
SQ_EPS = 1e-11
# exp-argument shifts (softmax is shift-invariant; keeps exp args small)
C1 = 29.5
C2 = 59.0

_CACHE = {}


def build_kernel(b_loc):
    n_tiles = b_loc // 128
    RG = min(8, n_tiles)           # tiles per routing group
    n_groups = (n_tiles + RG - 1) // RG
    assert n_tiles % RG == 0
    n_chunks = b_loc // 512 if b_loc >= 512 else 1
    chunk_rows = b_loc // n_chunks
    assert chunk_rows % 128 == 0
    n_bsub = chunk_rows // 128
    N_STAT = float(B_TOTAL_EFF[0] * NCAPS)

    nc = bacc.Bacc("TRN2", target_bir_lowering=False, debug=False, num_devices=NCORES)

    # ---------------- I/O ----------------
    x_d = nc.dram_tensor("x", [b_loc, DIM], dt.float32, kind="ExternalInput").ap()
    wfT_d = nc.dram_tensor("wfT", [128, 8, DIM], dt.float32r, kind="ExternalInput").ap()
    w1T_d = nc.dram_tensor("w1T", [128, 8, DIM], dt.bfloat16, kind="ExternalInput").ap()
    w2T_d = nc.dram_tensor("w2T", [128, 8, DIM], dt.bfloat16, kind="ExternalInput").ap()
    b1pe_d = nc.dram_tensor("b1pe", [128, 8], dt.float32, kind="ExternalInput").ap()
    b2row_d = nc.dram_tensor("b2row", [1, DIM], dt.float32, kind="ExternalInput").ap()
    gam2_d = nc.dram_tensor("gam2", [128, 2], dt.float32, kind="ExternalInput").ap()
    bet2_d = nc.dram_tensor("bet2", [128, 2], dt.float32, kind="ExternalInput").ap()
    out_d = nc.dram_tensor("out", [b_loc, DIM], dt.float32, kind="ExternalOutput").ap()

    with tile.TileContext(nc) as tc:
        with (
            tc.tile_pool(name="weights", bufs=1) as wp,
            tc.tile_pool(name="singles", bufs=1) as sp,
            tc.tile_pool(name="xin", bufs=2) as xp,
            tc.tile_pool(name="xt", bufs=3) as xtp,
            tc.tile_pool(name="u16", bufs=RG + 2) as up,
            tc.tile_pool(name="scratch", bufs=3) as scp,
            tc.tile_pool(name="routing", bufs=2) as rp,
            tc.tile_pool(name="vfT", bufs=3) as vtp,
            tc.tile_pool(name="hT", bufs=2) as htp,
            tc.tile_pool(name="outp", bufs=3) as op_,
            tc.tile_pool(name="ps", bufs=8, space="PSUM") as ps,
            tc.tile_pool(name="dram", bufs=1, space="DRAM") as dram,
        ):
            # ---------------- constants / weights ----------------
            wfT = wp.tile([128, 8, DIM], dt.float32r)
            nc.sync.dma_start(wfT[:], wfT_d[:])
            w1T = wp.tile([128, 8, DIM], dt.bfloat16)
            nc.sync.dma_start(w1T[:], w1T_d[:])
            w2T = wp.tile([128, 8, DIM], dt.bfloat16)
            nc.sync.dma_start(w2T[:], w2T_d[:])
            b1pe = sp.tile([128, 8], dt.float32)
            nc.sync.dma_start(b1pe[:], b1pe_d[:])
            b2bc = sp.tile([128, DIM], dt.float32)
            nc.sync.dma_start(b2bc[:], b2row_d[:].partition_broadcast(128).squeeze(1))
            gam2 = sp.tile([128, 2], dt.float32)
            nc.sync.dma_start(gam2[:], gam2_d[:])
            bet2 = sp.tile([128, 2], dt.float32)
            nc.sync.dma_start(bet2[:], bet2_d[:])

            ident = sp.tile([128, 128], dt.float32)
            make_identity(nc, ident)
            ones_f = sp.tile([128, 1], dt.float32)
            nc.vector.memset(ones_f[:], 1.0)
            ones_r = sp.tile([128, 1], dt.float32r)
            nc.vector.tensor_copy(ones_r[:], ones_f[:])
            ceps11 = sp.tile([128, 1], dt.float32)
            nc.vector.memset(ceps11[:], SQ_EPS)
            cepsbn = sp.tile([128, 1], dt.float32)
            nc.vector.memset(cepsbn[:], BN_EPS)
            cnc1 = sp.tile([128, 1], dt.float32)
            nc.vector.memset(cnc1[:], -C1)
            cnc2 = sp.tile([128, 1], dt.float32)
            nc.vector.memset(cnc2[:], -C2)

            acc1 = sp.tile([128, DIM], dt.float32)
            acc2 = sp.tile([128, DIM], dt.float32)
            nc.vector.memset(acc1[:], 0.0)
            nc.vector.memset(acc2[:], 0.0)
            usq_all = sp.tile([128, 4 * n_tiles], dt.float32)

            vf_dram = dram.tile([b_loc, DIM], dt.bfloat16)

            # ================= PHASE A =================
            vf_tiles = {}
            for g in range(n_groups):
                g_tiles = range(g * RG, (g + 1) * RG)
                for t in g_tiles:
                    x_sb = xp.tile([128, DIM], dt.float32, tag="x")
                    nc.sync.dma_start(x_sb[:], x_d[t * 128 : (t + 1) * 128, :])
                    # transpose x tile -> xT (fp32 PE transpose, 8 blocks, 2 psum banks)
                    xT = xtp.tile([128, 8, 128], dt.float32r, tag="xT")
                    for bh in range(2):
                        xt_ps = ps.tile([128, 4, 128], dt.float32, tag="bank", name=f"xt_{t}_{bh}")
                        for k in range(4):
                            kk = bh * 4 + k
                            nc.tensor.transpose(xt_ps[:, k, :], x_sb[:, kk * 128 : (kk + 1) * 128], ident[:])
                        nc.vector.tensor_copy(xT[:, bh * 4 : (bh + 1) * 4, :], xt_ps[:])
                    # L1 matmul: u[b, :] (batch-major), fp32r; capsule norms fused on ACT
                    u16 = up.tile([128, DIM], dt.bfloat16, tag="u16")
                    for dh in range(2):
                        u_ps = ps.tile([128, 512], dt.float32, tag="bank", name=f"u_{t}_{dh}")
                        for k in range(8):
                            nc.tensor.matmul(
                                u_ps[:],
                                xT[:, k, :],
                                wfT[:, k, dh * 512 : (dh + 1) * 512],
                                start=(k == 0), stop=(k == 7),
                            )
                        scr = scp.tile([128, 512], dt.float32, tag="scr", name=f"scr_{t}_{dh}")
                        for n in range(2):
                            nn_ = dh * 2 + n
                            nc.scalar.activation(
                                scr[:, n * DCAPS : (n + 1) * DCAPS],
                                u_ps[:, n * DCAPS : (n + 1) * DCAPS],
                                AF.Square,
                                accum_out=usq_all[:, 4 * t + nn_ : 4 * t + nn_ + 1],
                            )
                        nc.vector.tensor_copy(u16[:, dh * 512 : (dh + 1) * 512], u_ps[:])
                    vf_tiles[t] = u16

                # ---- routing for this group: alpha from usq ----
                nt = RG * 4
                usq = usq_all[:, g * nt : (g + 1) * nt]

                def view3(ap):
                    return ap.rearrange("p (g n) -> p g n", n=4)

                sq = rp.tile([128, nt], dt.float32, tag="r_sq")
                den = rp.tile([128, nt], dt.float32, tag="r_den")
                srt = rp.tile([128, nt], dt.float32, tag="r_srt")
                gg = rp.tile([128, nt], dt.float32, tag="r_g")
                bb = rp.tile([128, nt], dt.float32, tag="r_b")
                ee = rp.tile([128, nt], dt.float32, tag="r_e")
                ss = rp.tile([128, RG], dt.float32, tag="r_s")
                cc = rp.tile([128, nt], dt.float32, tag="r_c")
                uv = rp.tile([128, nt], dt.float32, tag="r_uv")
                al = rp.tile([128, nt], dt.float32, tag="r_al")

                def emit_g(sq_ap, out_ap):
                    # out = (sq/(1+sq)) / sqrt(sq + 1e-11)
                    nc.vector.tensor_scalar_add(den[:], sq_ap, 1.0)
                    nc.vector.reciprocal(den[:], den[:])
                    nc.scalar.activation(srt[:], sq_ap, AF.Sqrt, bias=ceps11[:])
                    nc.vector.reciprocal(srt[:], srt[:])
                    nc.vector.tensor_tensor(out_ap, sq_ap, den[:], ALU.mult)
                    nc.vector.tensor_tensor(out_ap, out_ap, srt[:], ALU.mult)

                def emit_softmax(b_ap, shift, out_ap):
                    # out = softmax over capsule groups of 4 (shifted exp)
                    nc.scalar.activation(ee[:], b_ap, AF.Exp, bias=shift[:])
                    nc.vector.tensor_reduce(ss[:], view3(ee[:]), axis=mybir.AxisListType.X, op=ALU.add)
                    nc.vector.reciprocal(ss[:], ss[:])
                    nc.vector.tensor_tensor(
                        view3(out_ap), view3(ee[:]),
                        ss[:, :, None].to_broadcast((128, RG, 4)), ALU.mult)

                # iter 1: c = 1/4 -> sq1 = usq/16; b = g(sq1) * usq * 0.25
                nc.vector.tensor_scalar_mul(sq[:], usq, 1.0 / 16.0)
                emit_g(sq[:], gg[:])
                nc.vector.tensor_tensor(bb[:], gg[:], usq, ALU.mult)
                nc.vector.tensor_scalar_mul(bb[:], bb[:], 0.25)
                # iter 2
                emit_softmax(bb[:], cnc1, cc[:])
                nc.vector.tensor_tensor(sq[:], cc[:], cc[:], ALU.mult)
                nc.vector.tensor_tensor(sq[:], sq[:], usq, ALU.mult)
                emit_g(sq[:], gg[:])
                nc.vector.tensor_tensor(uv[:], gg[:], cc[:], ALU.mult)
                nc.vector.tensor_tensor(uv[:], uv[:], usq, ALU.mult)
                nc.vector.tensor_tensor(bb[:], bb[:], uv[:], ALU.add)
                # final coefficients
                emit_softmax(bb[:], cnc2, cc[:])
                nc.vector.tensor_tensor(sq[:], cc[:], cc[:], ALU.mult)
                nc.vector.tensor_tensor(sq[:], sq[:], usq, ALU.mult)
                emit_g(sq[:], al[:])
                nc.vector.tensor_tensor(al[:], al[:], cc[:], ALU.mult)

                # ---- apply alpha, accumulate BN stats, spill vf ----
                for t in g_tiles:
                    u16 = vf_tiles.pop(t)
                    tl = t - g * RG
                    for n in range(NCAPS):
                        nc.vector.tensor_scalar_mul(
                            u16[:, n * DCAPS : (n + 1) * DCAPS],
                            u16[:, n * DCAPS : (n + 1) * DCAPS],
                            al[:, tl * 4 + n : tl * 4 + n + 1],
                        )
                    nc.vector.tensor_tensor(acc1[:], acc1[:], u16[:], ALU.add)
                    scr2 = scp.tile([128, DIM], dt.float32, tag="scr")
                    nc.scalar.activation(scr2[:], u16[:], AF.Square)
                    nc.vector.tensor_tensor(acc2[:], acc2[:], scr2[:], ALU.add)
                    nc.sync.dma_start(vf_dram[t * 128 : (t + 1) * 128, :], u16[:])

            # ================= BN STATS + ALLREDUCE =================
            accr = sp.tile([128, 2 * DIM], dt.float32r)
            nc.vector.tensor_copy(accr[:, 0:DIM], acc1[:])
            nc.vector.tensor_copy(accr[:, DIM : 2 * DIM], acc2[:])
            s_sb = sp.tile([1, 2 * DIM], dt.float32)
            for h in range(4):
                s_ps = ps.tile([1, 512], dt.float32, tag="bank", name=f"sfold_{h}")
                nc.tensor.matmul(
                    s_ps[:],
                    ones_r[:], accr[:, h * 512 : (h + 1) * 512],
                    start=True, stop=True)
                nc.vector.tensor_copy(s_sb[:, h * 512 : (h + 1) * 512], s_ps[:])
            cc_in = dram.tile([1, 2 * DIM], dt.float32)
            cc_out = dram.tile([1, 2 * DIM], dt.float32, addr_space="Shared")
            nc.sync.dma_start(cc_in[:], s_sb[:])
            nc.gpsimd.collective_compute(
                "AllReduce", ALU.add,
                ins=[cc_in[:].opt()], outs=[cc_out[:].opt()],
                replica_groups=[list(range(NCORES))])
            # load global sums partition-scattered: Sg[p, o] = cc_out[o*128+p]
            Sg = sp.tile([128, 16], dt.float32)
            nc.sync.dma_start(Sg[:], cc_out[:].squeeze(0).rearrange("(o p) -> p o", p=128))
            # fold over capsules: dc = tt*128+p ; S1c[:, tt] = sum_n Sg[:, 2n+tt]
            S1c = sp.tile([128, 2], dt.float32)
            S2c = sp.tile([128, 2], dt.float32)
            for tt in range(2):
                nc.vector.tensor_reduce(
                    S1c[:, tt : tt + 1], Sg[:, tt : 8 : 2],
                    axis=mybir.AxisListType.X, op=ALU.add)
                nc.vector.tensor_reduce(
                    S2c[:, tt : tt + 1], Sg[:, 8 + tt : 16 : 2],
                    axis=mybir.AxisListType.X, op=ALU.add)
            mean = sp.tile([128, 2], dt.float32)
            var = sp.tile([128, 2], dt.float32)
            a2 = sp.tile([128, 2], dt.float32)
            m2 = sp.tile([128, 2], dt.float32)
            nc.vector.tensor_scalar_mul(mean[:], S1c[:], 1.0 / N_STAT)
            nc.vector.tensor_scalar_mul(var[:], S2c[:], 1.0 / N_STAT)
            tmp22 = sp.tile([128, 2], dt.float32)
            nc.vector.tensor_tensor(tmp22[:], mean[:], mean[:], ALU.mult)
            nc.vector.tensor_tensor(var[:], var[:], tmp22[:], ALU.subtract)
            nc.scalar.activation(var[:], var[:], AF.Sqrt, bias=cepsbn[:])
            nc.vector.reciprocal(var[:], var[:])          # rstd
            nc.vector.tensor_tensor(a2[:], gam2[:], var[:], ALU.mult)
            nc.vector.tensor_tensor(m2[:], mean[:], a2[:], ALU.mult)
            nc.vector.tensor_tensor(m2[:], bet2[:], m2[:], ALU.subtract)
            m2b = sp.tile([128, 2], dt.bfloat16)
            nc.vector.tensor_copy(m2b[:], m2[:])

            # b1eff[e] = b1[e] + sum_j W1[e, j] * m[j]  (before scaling W1T!)
            b1eff = sp.tile([128, 8], dt.float32)
            for ec in range(8):
                be_ps = ps.tile([128, 1], dt.float32, tag="bank", name=f"b1e_{ec}")
                for k in range(8):
                    nc.tensor.matmul(
                        be_ps[:], w1T[:, k, ec * 128 : (ec + 1) * 128],
                        m2b[:, k % 2 : k % 2 + 1],
                        start=(k == 0), stop=(k == 7))
                nc.vector.tensor_tensor(
                    b1eff[:, ec : ec + 1], be_ps[:], b1pe[:, ec : ec + 1], ALU.add)
            # scale W1T rows by a (j on partitions) in place
            for k in range(8):
                nc.vector.tensor_scalar_mul(
                    w1T[:, k, :], w1T[:, k, :], a2[:, k % 2 : k % 2 + 1])

            # ================= PHASE B: MLP =================
            for c in range(n_chunks):
                vfT = vtp.tile([128, 8, chunk_rows], dt.bfloat16, tag="vfT")
                nc.sync.dma_start_transpose(
                    vfT[:], vf_dram[c * chunk_rows : (c + 1) * chunk_rows, :])
                hT = htp.tile([128, 8, chunk_rows], dt.bfloat16, tag="hT")
                for ec in range(8):
                    h_ps = ps.tile([128, chunk_rows], dt.float32, tag="bank", name=f"h_{c}_{ec}")
                    for k in range(8):
                        nc.tensor.matmul(
                            h_ps[:], w1T[:, k, ec * 128 : (ec + 1) * 128],
                            vfT[:, k, :], start=(k == 0), stop=(k == 7))
                    nc.scalar.activation(
                        hT[:, ec, :], h_ps[:], AF.Relu, bias=b1eff[:, ec : ec + 1])
                for bs in range(n_bsub):
                    o_sb = op_.tile([128, DIM], dt.float32, tag="osb")
                    for ot in range(2):
                        o_ps = ps.tile([128, 512], dt.float32, tag="bank", name=f"o_{c}_{bs}_{ot}")
                        for k in range(8):
                            nc.tensor.matmul(
                                o_ps[:], hT[:, k, bs * 128 : (bs + 1) * 128],
                                w2T[:, k, ot * 512 : (ot + 1) * 512],
                                start=(k == 0), stop=(k == 7))
                        nc.vector.tensor_tensor(
                            o_sb[:, ot * 512 : (ot + 1) * 512], o_ps[:],
                            b2bc[:, ot * 512 : (ot + 1) * 512], ALU.add)
                    r0 = c * chunk_rows + bs * 128
                    nc.sync.dma_start(out_d[r0 : r0 + 128, :], o_sb[:])

    nc.compile()
    return nc


# effective total batch for BN statistics (mutable for small-scale testing)
B_TOTAL_EFF = [B_TOTAL]


def _prep_weights(W, gamma, beta, W1, b1, W2, b2):
    bf16 = ml_dtypes.bfloat16
    Wf = np.ascontiguousarray(W.reshape(DIM, DIM))           # [j, i]
    wfT = np.ascontiguousarray(Wf.T.reshape(8, 128, DIM).transpose(1, 0, 2))  # [p, k, j]
    w1T = np.ascontiguousarray(W1.T.reshape(8, 128, DIM).transpose(1, 0, 2)).astype(bf16)
    w2T = np.ascontiguousarray(W2.T.reshape(8, 128, DIM).transpose(1, 0, 2)).astype(bf16)
    b1pe = np.ascontiguousarray(b1.reshape(8, 128).T).astype(np.float32)
    b2row = np.ascontiguousarray(b2.reshape(1, DIM)).astype(np.float32)
    gam2 = np.ascontiguousarray(gamma.reshape(2, 128).T).astype(np.float32)
    bet2 = np.ascontiguousarray(beta.reshape(2, 128).T).astype(np.float32)
    return dict(wfT=wfT.astype(np.float32), w1T=w1T, w2T=w2T,
                b1pe=b1pe, b2row=b2row, gam2=gam2, bet2=bet2)


def kernel(x, W, gamma, beta, W1, b1, W2, b2):
    x = np.asarray(x, np.float32)
    B = x.shape[0]
    b_loc = B // NCORES
    B_TOTAL_EFF[0] = B
    key = (b_loc, B)
    if key not in _CACHE:
        _CACHE[key] = build_kernel(b_loc)
    nc = _CACHE[key]
    wmaps = _prep_weights(np.asarray(W, np.float32), np.asarray(gamma, np.float32),
                          np.asarray(beta, np.float32), np.asarray(W1, np.float32),
                          np.asarray(b1, np.float32), np.asarray(W2, np.float32),
                          np.asarray(b2, np.float32))
    in_maps = [dict(wmaps, x=np.ascontiguousarray(x[i * b_loc : (i + 1) * b_loc]))
               for i in range(NCORES)]
    res = run_bass_kernel_spmd(nc, in_maps, core_ids=list(range(NCORES)))
    out = np.concatenate([res.results[i]["out"] for i in range(NCORES)], axis=0)
    return out


# revision 16
# speedup vs baseline: 1.4997x; 1.4997x over previous
"""Trainium2 Bass kernel for nn_CapsuleNetwork (self-contained).

Math (reference, with IN_CAPS == 1):
  u = x @ Wf.T                      # [B, 1024], Wf = W.reshape(1024, 1024)
  usq[b,n] = sum_d u[b, n*256+d]^2  # capsule norms
  routing (2 iters) collapses to scalar math on usq -> alpha[b,n]
  v = alpha * u (per capsule)       # [B, 4, 256]
  batchnorm over (B, n) per channel dc -> fold into W1
  out = relu(v_bn @ W1.T + b1) @ W2.T + b2

Distribution: data-parallel over batch across 8 cores; BN stats all-reduced.

Precision: layer-1 matmul in float32r (hw ~1.4e-4), routing math fp32,
v stored bf16, MLP in bf16, output fp32.
"""
import numpy as np
import ml_dtypes

import concourse.bass as bass
import concourse.mybir as mybir
import concourse.tile as tile
from concourse import bacc
from concourse.bass_utils import run_bass_kernel_spmd

dt = mybir.dt
AF = mybir.ActivationFunctionType
ALU = mybir.AluOpType

NCORES = 8
B_TOTAL = 32768
DIM = 1024
NCAPS = 4
DCAPS = 256
BN_EPS = 1e-5
SQ_EPS = 1e-11
# exp-argument shifts (softmax is shift-invariant; keeps exp args small)
C1 = 29.5
C2 = 59.0

_CACHE = {}


def build_kernel(b_loc):
    n_tiles = b_loc // 128
    RG = min(8, n_tiles)           # tiles per routing group
    n_groups = (n_tiles + RG - 1) // RG
    assert n_tiles % RG == 0
    n_chunks = b_loc // 512 if b_loc >= 512 else 1
    chunk_rows = b_loc // n_chunks
    assert chunk_rows % 128 == 0
    n_bsub = chunk_rows // 128
    N_STAT = float(B_TOTAL_EFF[0] * NCAPS)

    nc = bacc.Bacc("TRN2", target_bir_lowering=False, debug=False, num_devices=NCORES)

    # ---------------- I/O ----------------
    x_d = nc.dram_tensor("x", [b_loc, DIM], dt.float32, kind="ExternalInput").ap()
    wfT_d = nc.dram_tensor("wfT", [128, 8, DIM], dt.float32r, kind="ExternalInput").ap()
    w1T_d = nc.dram_tensor("w1T", [128, 8, DIM], dt.bfloat16, kind="ExternalInput").ap()
    w2T_d = nc.dram_tensor("w2T", [128, 8, DIM], dt.bfloat16, kind="ExternalInput").ap()
    b1pe_d = nc.dram_tensor("b1pe", [128, 8], dt.float32, kind="ExternalInput").ap()
    b2row_d = nc.dram_tensor("b2row", [1, DIM], dt.float32, kind="ExternalInput").ap()
    gam2_d = nc.dram_tensor("gam2", [128, 2], dt.float32, kind="ExternalInput").ap()
    bet2_d = nc.dram_tensor("bet2", [128, 2], dt.float32, kind="ExternalInput").ap()
    ident_d = nc.dram_tensor("ident", [128, 128], dt.float32, kind="ExternalInput").ap()
    out_d = nc.dram_tensor("out", [b_loc, DIM], dt.float32, kind="ExternalOutput").ap()

    with tile.TileContext(nc) as tc:
        with (
            tc.tile_pool(name="weights", bufs=1) as wp,
            tc.tile_pool(name="singles", bufs=1) as sp,
            tc.tile_pool(name="xin", bufs=2) as xp,
            tc.tile_pool(name="xt", bufs=3) as xtp,
            tc.tile_pool(name="u16", bufs=2 * RG + 1) as up,
            tc.tile_pool(name="scratch", bufs=2) as scp,
            tc.tile_pool(name="routing", bufs=2) as rp,
            tc.tile_pool(name="vfT", bufs=2) as vtp,
            tc.tile_pool(name="hT", bufs=2) as htp,
            tc.tile_pool(name="outp", bufs=2) as op_,
            tc.tile_pool(name="ps", bufs=8, space="PSUM") as ps,
            tc.tile_pool(name="dram", bufs=1, space="DRAM") as dram,
        ):
            # ---------------- constants / weights ----------------
            wfT = wp.tile([128, 8, DIM], dt.float32r)
            for k in range(8):
                nc.gpsimd.dma_start(wfT[:, k, :], wfT_d[:, k, :])
            w1T = wp.tile([128, 8, DIM], dt.bfloat16)
            nc.gpsimd.dma_start(w1T[:], w1T_d[:])
            w2T = wp.tile([128, 8, DIM], dt.bfloat16)
            nc.gpsimd.dma_start(w2T[:], w2T_d[:])
            b1pe = sp.tile([128, 8], dt.float32)
            nc.gpsimd.dma_start(b1pe[:], b1pe_d[:])
            b2bc = sp.tile([128, DIM], dt.float32)
            nc.gpsimd.dma_start(b2bc[:], b2row_d[:].partition_broadcast(128).squeeze(1))
            gam2 = sp.tile([128, 2], dt.float32)
            nc.gpsimd.dma_start(gam2[:], gam2_d[:])
            bet2 = sp.tile([128, 2], dt.float32)
            nc.gpsimd.dma_start(bet2[:], bet2_d[:])

            ident = sp.tile([128, 128], dt.float32)
            nc.sync.dma_start(ident[:], ident_d[:])
            ones_f = sp.tile([128, 1], dt.float32)
            nc.vector.memset(ones_f[:], 1.0)
            ones_r = sp.tile([128, 1], dt.float32r)
            nc.vector.tensor_copy(ones_r[:], ones_f[:])
            ceps11 = sp.tile([128, 1], dt.float32)
            nc.vector.memset(ceps11[:], SQ_EPS)
            cepsbn = sp.tile([128, 1], dt.float32)
            nc.vector.memset(cepsbn[:], BN_EPS)
            cnc1 = sp.tile([128, 1], dt.float32)
            nc.vector.memset(cnc1[:], -C1)
            cnc2 = sp.tile([128, 1], dt.float32)
            nc.vector.memset(cnc2[:], -C2)

            acc1 = sp.tile([128, DIM], dt.float32)
            acc2 = sp.tile([128, DIM], dt.float32)
            nc.vector.memset(acc1[:], 0.0)
            nc.vector.memset(acc2[:], 0.0)

            vf_dram = dram.tile([b_loc, DIM], dt.bfloat16)

            # ================= PHASE A =================
            # Software-pipelined emission: group g's routing/apply is emitted
            # after group g+1's L1 tiles so engine FIFOs let the next group's
            # PSUM evacuations run ahead of the serial routing chain.
            vf_tiles = {}
            usq_tiles = {}

            def emit_l1_tiles(g):
                g_tiles = range(g * RG, (g + 1) * RG)
                usq_g = rp.tile([128, 4 * RG], dt.float32, tag="usq", bufs=2, name=f"usq_{g}")
                usq_tiles[g] = usq_g
                for t in g_tiles:
                    x_sb = xp.tile([128, DIM], dt.float32, tag="x", name=f"x_{t}")
                    nc.sync.dma_start(x_sb[:], x_d[t * 128 : (t + 1) * 128, :])
                    # transpose x tile -> xT (fp32 PE transpose, 8 blocks, 2 psum banks)
                    xT = xtp.tile([128, 8, 128], dt.float32r, tag="xT", name=f"xT_{t}")
                    for bh in range(2):
                        xt_ps = ps.tile([128, 4, 128], dt.float32, tag="bank", name=f"xt_{t}_{bh}")
                        for k in range(4):
                            kk = bh * 4 + k
                            nc.tensor.transpose(xt_ps[:, k, :], x_sb[:, kk * 128 : (kk + 1) * 128], ident[:])
                        nc.vector.tensor_copy(xT[:, bh * 4 : (bh + 1) * 4, :], xt_ps[:])
                    # L1 matmul: u[b, :] (batch-major), fp32r; capsule norms fused on ACT
                    u16 = up.tile([128, DIM], dt.bfloat16, tag="u16", name=f"u16_{t}")
                    for dh in range(2):
                        u_ps = ps.tile([128, 512], dt.float32, tag="bank", name=f"u_{t}_{dh}")
                        for k in range(8):
                            nc.tensor.matmul(
                                u_ps[:],
                                xT[:, k, :],
                                wfT[:, k, dh * 512 : (dh + 1) * 512],
                                start=(k == 0), stop=(k == 7),
                            )
                        scr = scp.tile([128, 512], dt.float32, tag="scr", name=f"scr_{t}_{dh}")
                        for n in range(2):
                            nn_ = dh * 2 + n
                            nc.scalar.activation(
                                scr[:, n * DCAPS : (n + 1) * DCAPS],
                                u_ps[:, n * DCAPS : (n + 1) * DCAPS],
                                AF.Square,
                                accum_out=usq_g[:, 4 * (t - g * RG) + nn_ : 4 * (t - g * RG) + nn_ + 1],
                            )
                        nc.scalar.copy(u16[:, dh * 512 : (dh + 1) * 512], u_ps[:])
                    vf_tiles[t] = u16

            def emit_routing_apply(g):
                g_tiles = range(g * RG, (g + 1) * RG)
                usq = usq_tiles.pop(g)[:]
                nt = RG * 4

                def view3(ap):
                    return ap.rearrange("p (g n) -> p g n", n=4)

                sq = rp.tile([128, nt], dt.float32, tag="r_sq", name=f"sq_{g}")
                den = rp.tile([128, nt], dt.float32, tag="r_den", name=f"den_{g}")
                gg = rp.tile([128, nt], dt.float32, tag="r_g", name=f"gg_{g}")
                bb = rp.tile([128, nt], dt.float32, tag="r_b", name=f"bb_{g}")
                ee = rp.tile([128, nt], dt.float32, tag="r_e", name=f"ee_{g}")
                ss = rp.tile([128, RG], dt.float32, tag="r_s", name=f"ss_{g}")
                cc = rp.tile([128, nt], dt.float32, tag="r_c", name=f"cc_{g}")
                uv = rp.tile([128, nt], dt.float32, tag="r_uv", name=f"uv_{g}")
                al = rp.tile([128, nt], dt.float32, tag="r_al", name=f"al_{g}")

                def emit_softmax(b_ap, shift, out_ap):
                    # out = softmax over capsule groups of 4 (shifted exp)
                    nc.scalar.activation(ee[:], b_ap, AF.Exp, bias=shift[:])
                    nc.vector.tensor_reduce(ss[:], view3(ee[:]), axis=mybir.AxisListType.X, op=ALU.add)
                    nc.vector.reciprocal(ss[:], ss[:])
                    nc.vector.tensor_tensor(
                        view3(out_ap), view3(ee[:]),
                        ss[:, :, None].to_broadcast((128, RG, 4)), ALU.mult)

                # rt = sqrt(usq); with t = c^2*usq:
                #   uv    = rt * t/(1+t)
                #   alpha = c^2 * rt / (1+t)
                rt = rp.tile([128, nt], dt.float32, tag="r_rt", name=f"rt_{g}")
                c2t = rp.tile([128, nt], dt.float32, tag="r_c2", name=f"c2_{g}")
                nc.scalar.activation(rt[:], usq, AF.Sqrt, bias=ceps11[:])
                # iter 1 (c = 1/4 -> t = usq/16)
                nc.vector.tensor_scalar_mul(sq[:], usq, 1.0 / 16.0)
                nc.vector.tensor_scalar_add(den[:], sq[:], 1.0)
                nc.vector.reciprocal(den[:], den[:])
                nc.vector.tensor_tensor(gg[:], sq[:], den[:], ALU.mult)
                nc.vector.tensor_tensor(bb[:], gg[:], rt[:], ALU.mult)
                # iter 2
                emit_softmax(bb[:], cnc1, cc[:])
                nc.scalar.activation(c2t[:], cc[:], AF.Square)
                nc.vector.tensor_tensor(sq[:], c2t[:], usq, ALU.mult)
                nc.vector.tensor_scalar_add(den[:], sq[:], 1.0)
                nc.vector.reciprocal(den[:], den[:])
                nc.vector.tensor_tensor(gg[:], sq[:], den[:], ALU.mult)
                nc.vector.tensor_tensor(uv[:], gg[:], rt[:], ALU.mult)
                nc.vector.tensor_tensor(bb[:], bb[:], uv[:], ALU.add)
                # final coefficients
                emit_softmax(bb[:], cnc2, cc[:])
                nc.scalar.activation(c2t[:], cc[:], AF.Square)
                nc.vector.tensor_tensor(sq[:], c2t[:], usq, ALU.mult)
                nc.vector.tensor_scalar_add(den[:], sq[:], 1.0)
                nc.vector.reciprocal(den[:], den[:])
                nc.vector.tensor_tensor(gg[:], rt[:], den[:], ALU.mult)
                nc.vector.tensor_tensor(al[:], gg[:], c2t[:], ALU.mult)

                # ---- apply alpha (split DVE/ACT), accumulate BN stats, spill vf ----
                for t in g_tiles:
                    u16 = vf_tiles.pop(t)
                    tl = t - g * RG
                    for n in range(NCAPS):
                        nc.vector.tensor_scalar_mul(
                            u16[:, n * DCAPS : (n + 1) * DCAPS],
                            u16[:, n * DCAPS : (n + 1) * DCAPS],
                            al[:, tl * 4 + n : tl * 4 + n + 1])
                    nc.vector.tensor_tensor(acc1[:], acc1[:], u16[:], ALU.add)
                    scr2 = scp.tile([128, DIM], dt.float32, tag="scr2", name=f"scr2_{t}")
                    nc.scalar.activation(scr2[:], u16[:], AF.Square)
                    nc.gpsimd.tensor_tensor(acc2[:], acc2[:], scr2[:], ALU.add)
                    nc.sync.dma_start(vf_dram[t * 128 : (t + 1) * 128, :], u16[:])

            for g in range(n_groups):
                emit_l1_tiles(g)
                if g >= 1:
                    emit_routing_apply(g - 1)
            emit_routing_apply(n_groups - 1)

            # ================= BN STATS + ALLREDUCE =================
            accr = sp.tile([128, 2 * DIM], dt.float32r)
            nc.vector.tensor_copy(accr[:, 0:DIM], acc1[:])
            nc.vector.tensor_copy(accr[:, DIM : 2 * DIM], acc2[:])
            s_sb = sp.tile([1, 2 * DIM], dt.float32)
            for h in range(4):
                s_ps = ps.tile([1, 512], dt.float32, tag="bank", name=f"sfold_{h}")
                nc.tensor.matmul(
                    s_ps[:],
                    ones_r[:], accr[:, h * 512 : (h + 1) * 512],
                    start=True, stop=True)
                nc.vector.tensor_copy(s_sb[:, h * 512 : (h + 1) * 512], s_ps[:])
            cc_in = dram.tile([1, 2 * DIM], dt.float32)
            cc_out = dram.tile([1, 2 * DIM], dt.float32, addr_space="Shared")
            nc.sync.dma_start(cc_in[:], s_sb[:])
            nc.gpsimd.collective_compute(
                "AllReduce", ALU.add,
                ins=[cc_in[:].opt()], outs=[cc_out[:].opt()],
                replica_groups=[list(range(NCORES))])
            # load global sums partition-scattered: Sg[p, o] = cc_out[o*128+p]
            Sg = sp.tile([128, 16], dt.float32)
            nc.sync.dma_start(Sg[:], cc_out[:].squeeze(0).rearrange("(o p) -> p o", p=128))
            # fold over capsules: dc = tt*128+p ; S1c[:, tt] = sum_n Sg[:, 2n+tt]
            S1c = sp.tile([128, 2], dt.float32)
            S2c = sp.tile([128, 2], dt.float32)
            for tt in range(2):
                nc.vector.tensor_reduce(
                    S1c[:, tt : tt + 1], Sg[:, tt : 8 : 2],
                    axis=mybir.AxisListType.X, op=ALU.add)
                nc.vector.tensor_reduce(
                    S2c[:, tt : tt + 1], Sg[:, 8 + tt : 16 : 2],
                    axis=mybir.AxisListType.X, op=ALU.add)
            mean = sp.tile([128, 2], dt.float32)
            var = sp.tile([128, 2], dt.float32)
            a2 = sp.tile([128, 2], dt.float32)
            m2 = sp.tile([128, 2], dt.float32)
            nc.vector.tensor_scalar_mul(mean[:], S1c[:], 1.0 / N_STAT)
            nc.vector.tensor_scalar_mul(var[:], S2c[:], 1.0 / N_STAT)
            tmp22 = sp.tile([128, 2], dt.float32)
            nc.vector.tensor_tensor(tmp22[:], mean[:], mean[:], ALU.mult)
            nc.vector.tensor_tensor(var[:], var[:], tmp22[:], ALU.subtract)
            nc.scalar.activation(var[:], var[:], AF.Sqrt, bias=cepsbn[:])
            nc.vector.reciprocal(var[:], var[:])          # rstd
            nc.vector.tensor_tensor(a2[:], gam2[:], var[:], ALU.mult)
            nc.vector.tensor_tensor(m2[:], mean[:], a2[:], ALU.mult)
            nc.vector.tensor_tensor(m2[:], bet2[:], m2[:], ALU.subtract)
            m2b = sp.tile([128, 2], dt.bfloat16)
            nc.vector.tensor_copy(m2b[:], m2[:])

            # b1eff[e] = b1[e] + sum_j W1[e, j] * m[j], computed as a row
            # [1, 1024] via wide matmuls, then partition-scattered via DRAM.
            b1m_row = sp.tile([1, DIM], dt.float32)
            for h in range(2):
                br_ps = ps.tile([1, 512], dt.float32, tag="bank", name=f"b1r_{h}")
                for k in range(8):
                    nc.tensor.matmul(
                        br_ps[:], m2b[:, k % 2 : k % 2 + 1],
                        w1T[:, k, h * 512 : (h + 1) * 512],
                        start=(k == 0), stop=(k == 7))
                nc.vector.tensor_copy(b1m_row[:, h * 512 : (h + 1) * 512], br_ps[:])
            b1m_dram = dram.tile([1, DIM], dt.float32)
            nc.sync.dma_start(b1m_dram[:], b1m_row[:])
            b1m_pe = sp.tile([128, 8], dt.float32)
            nc.sync.dma_start(b1m_pe[:], b1m_dram[:].squeeze(0).rearrange("(k p) -> p k", p=128))
            b1eff = sp.tile([128, 8], dt.float32)
            nc.vector.tensor_tensor(b1eff[:], b1pe[:], b1m_pe[:], ALU.add)

            # ================= PHASE B: MLP =================
            for c in range(n_chunks):
                vfT = vtp.tile([128, 8, chunk_rows], dt.bfloat16, tag="vfT")
                nc.sync.dma_start_transpose(
                    vfT[:], vf_dram[c * chunk_rows : (c + 1) * chunk_rows, :])
                # BN scale applied to activations (a is per-j = per-partition)
                for k in range(8):
                    nc.vector.tensor_scalar_mul(
                        vfT[:, k, :], vfT[:, k, :], a2[:, k % 2 : k % 2 + 1])
                hT = htp.tile([128, 8, chunk_rows], dt.bfloat16, tag="hT")
                for ec in range(8):
                    h_ps = ps.tile([128, chunk_rows], dt.float32, tag="bank", name=f"h_{c}_{ec}")
                    for k in range(8):
                        nc.tensor.matmul(
                            h_ps[:], w1T[:, k, ec * 128 : (ec + 1) * 128],
                            vfT[:, k, :], start=(k == 0), stop=(k == 7))
                    nc.scalar.activation(
                        hT[:, ec, :], h_ps[:], AF.Relu, bias=b1eff[:, ec : ec + 1])
                for bs in range(n_bsub):
                    o_sb = op_.tile([128, DIM], dt.float32, tag="osb")
                    for ot in range(2):
                        o_ps = ps.tile([128, 512], dt.float32, tag="bank", name=f"o_{c}_{bs}_{ot}")
                        for k in range(8):
                            nc.tensor.matmul(
                                o_ps[:], hT[:, k, bs * 128 : (bs + 1) * 128],
                                w2T[:, k, ot * 512 : (ot + 1) * 512],
                                start=(k == 0), stop=(k == 7))
                        nc.vector.tensor_tensor(
                            o_sb[:, ot * 512 : (ot + 1) * 512], o_ps[:],
                            b2bc[:, ot * 512 : (ot + 1) * 512], ALU.add)
                    r0 = c * chunk_rows + bs * 128
                    nc.gpsimd.dma_start(out_d[r0 : r0 + 128, :], o_sb[:])

    nc.compile()
    return nc


# effective total batch for BN statistics (mutable for small-scale testing)
B_TOTAL_EFF = [B_TOTAL]


def _prep_weights(W, gamma, beta, W1, b1, W2, b2):
    bf16 = ml_dtypes.bfloat16
    Wf = np.ascontiguousarray(W.reshape(DIM, DIM))           # [j, i]
    wfT = np.ascontiguousarray(Wf.T.reshape(8, 128, DIM).transpose(1, 0, 2))  # [p, k, j]
    w1T = np.ascontiguousarray(W1.T.reshape(8, 128, DIM).transpose(1, 0, 2)).astype(bf16)
    w2T = np.ascontiguousarray(W2.T.reshape(8, 128, DIM).transpose(1, 0, 2)).astype(bf16)
    b1pe = np.ascontiguousarray(b1.reshape(8, 128).T).astype(np.float32)
    b2row = np.ascontiguousarray(b2.reshape(1, DIM)).astype(np.float32)
    gam2 = np.ascontiguousarray(gamma.reshape(2, 128).T).astype(np.float32)
    bet2 = np.ascontiguousarray(beta.reshape(2, 128).T).astype(np.float32)
    return dict(wfT=wfT.astype(np.float32), w1T=w1T, w2T=w2T,
                b1pe=b1pe, b2row=b2row, gam2=gam2, bet2=bet2,
                ident=np.eye(128, dtype=np.float32))


def kernel(x, W, gamma, beta, W1, b1, W2, b2):
    x = np.asarray(x, np.float32)
    B = x.shape[0]
    b_loc = B // NCORES
    B_TOTAL_EFF[0] = B
    key = (b_loc, B)
    if key not in _CACHE:
        _CACHE[key] = build_kernel(b_loc)
    nc = _CACHE[key]
    wmaps = _prep_weights(np.asarray(W, np.float32), np.asarray(gamma, np.float32),
                          np.asarray(beta, np.float32), np.asarray(W1, np.float32),
                          np.asarray(b1, np.float32), np.asarray(W2, np.float32),
                          np.asarray(b2, np.float32))
    in_maps = [dict(wmaps, x=np.ascontiguousarray(x[i * b_loc : (i + 1) * b_loc]))
               for i in range(NCORES)]
    res = run_bass_kernel_spmd(nc, in_maps, core_ids=list(range(NCORES)))
    out = np.concatenate([res.results[i]["out"] for i in range(NCORES)], axis=0)
    return out


# revision 18
# speedup vs baseline: 1.5228x; 1.0154x over previous
"""Trainium2 Bass kernel for nn_CapsuleNetwork (self-contained).

Math (reference, with IN_CAPS == 1):
  u = x @ Wf.T                      # [B, 1024], Wf = W.reshape(1024, 1024)
  usq[b,n] = sum_d u[b, n*256+d]^2  # capsule norms
  routing (2 iters) collapses to scalar math on usq -> alpha[b,n]
  v = alpha * u (per capsule)       # [B, 4, 256]
  batchnorm over (B, n) per channel dc -> fold into W1
  out = relu(v_bn @ W1.T + b1) @ W2.T + b2

Distribution: data-parallel over batch across 8 cores; BN stats all-reduced.

Precision: layer-1 matmul in float32r (hw ~1.4e-4), routing math fp32,
v stored bf16, MLP in bf16, output fp32.
"""
import numpy as np
import ml_dtypes

import concourse.bass as bass
import concourse.mybir as mybir
import concourse.tile as tile
from concourse import bacc
from concourse.bass_utils import run_bass_kernel_spmd

dt = mybir.dt
AF = mybir.ActivationFunctionType
ALU = mybir.AluOpType

NCORES = 8
B_TOTAL = 32768
DIM = 1024
NCAPS = 4
DCAPS = 256
BN_EPS = 1e-5
SQ_EPS = 1e-11
# exp-argument shifts (softmax is shift-invariant; keeps exp args small)
C1 = 29.5
C2 = 59.0

_CACHE = {}


def build_kernel(b_loc):
    n_tiles = b_loc // 128
    RG = min(8, n_tiles)           # tiles per routing group
    assert n_tiles % RG == 0
    groups = [RG] * (n_tiles // RG)
    g_starts = [sum(groups[:i]) for i in range(len(groups))]
    n_groups = len(groups)
    n_chunks = b_loc // 512 if b_loc >= 512 else 1
    chunk_rows = b_loc // n_chunks
    assert chunk_rows % 128 == 0
    n_bsub = chunk_rows // 128
    N_STAT = float(B_TOTAL_EFF[0] * NCAPS)

    nc = bacc.Bacc("TRN2", target_bir_lowering=False, debug=False, num_devices=NCORES)

    # ---------------- I/O ----------------
    x_d = nc.dram_tensor("x", [b_loc, DIM], dt.float32, kind="ExternalInput").ap()
    wfT_d = nc.dram_tensor("wfT", [128, 8, DIM], dt.float32r, kind="ExternalInput").ap()
    w1T_d = nc.dram_tensor("w1T", [128, 8, DIM], dt.bfloat16, kind="ExternalInput").ap()
    w2T_d = nc.dram_tensor("w2T", [128, 8, DIM], dt.bfloat16, kind="ExternalInput").ap()
    b1pe_d = nc.dram_tensor("b1pe", [128, 8], dt.float32, kind="ExternalInput").ap()
    b2row_d = nc.dram_tensor("b2row", [1, DIM], dt.float32, kind="ExternalInput").ap()
    gam2_d = nc.dram_tensor("gam2", [128, 2], dt.float32, kind="ExternalInput").ap()
    bet2_d = nc.dram_tensor("bet2", [128, 2], dt.float32, kind="ExternalInput").ap()
    ident_d = nc.dram_tensor("ident", [128, 128], dt.float32, kind="ExternalInput").ap()
    out_d = nc.dram_tensor("out", [b_loc, DIM], dt.float32, kind="ExternalOutput").ap()

    with tile.TileContext(nc) as tc:
        with (
            tc.tile_pool(name="weights", bufs=1) as wp,
            tc.tile_pool(name="singles", bufs=1) as sp,
            tc.tile_pool(name="xin", bufs=3) as xp,
            tc.tile_pool(name="xt", bufs=3) as xtp,
            tc.tile_pool(name="u16", bufs=2 * RG + 1) as up,
            tc.tile_pool(name="scratch", bufs=2) as scp,
            tc.tile_pool(name="routing", bufs=2) as rp,
            tc.tile_pool(name="vfT", bufs=2) as vtp,
            tc.tile_pool(name="hT", bufs=2) as htp,
            tc.tile_pool(name="outp", bufs=2) as op_,
            tc.tile_pool(name="ps", bufs=8, space="PSUM") as ps,
            tc.tile_pool(name="dram", bufs=1, space="DRAM") as dram,
        ):
            # ---------------- constants / weights ----------------
            wfT = wp.tile([128, 8, DIM], dt.float32r)
            for k in range(8):
                nc.gpsimd.dma_start(wfT[:, k, :], wfT_d[:, k, :])
            w1T = wp.tile([128, 8, DIM], dt.bfloat16)
            nc.gpsimd.dma_start(w1T[:], w1T_d[:])
            w2T = wp.tile([128, 8, DIM], dt.bfloat16)
            nc.gpsimd.dma_start(w2T[:], w2T_d[:])
            b1pe = sp.tile([128, 8], dt.float32)
            nc.gpsimd.dma_start(b1pe[:], b1pe_d[:])
            b2bc = sp.tile([128, DIM], dt.float32)
            nc.gpsimd.dma_start(b2bc[:], b2row_d[:].partition_broadcast(128).squeeze(1))
            gam2 = sp.tile([128, 2], dt.float32)
            nc.gpsimd.dma_start(gam2[:], gam2_d[:])
            bet2 = sp.tile([128, 2], dt.float32)
            nc.gpsimd.dma_start(bet2[:], bet2_d[:])

            ident = sp.tile([128, 128], dt.float32)
            nc.sync.dma_start(ident[:], ident_d[:])
            ones_f = sp.tile([128, 1], dt.float32)
            nc.vector.memset(ones_f[:], 1.0 / N_STAT)
            ones_r = sp.tile([128, 1], dt.float32r)
            nc.vector.tensor_copy(ones_r[:], ones_f[:])
            ceps11 = sp.tile([128, 1], dt.float32)
            nc.vector.memset(ceps11[:], SQ_EPS)
            cepsbn = sp.tile([128, 1], dt.float32)
            nc.vector.memset(cepsbn[:], BN_EPS)
            cnc1 = sp.tile([128, 1], dt.float32)
            nc.vector.memset(cnc1[:], -C1)
            cnc2 = sp.tile([128, 1], dt.float32)
            nc.vector.memset(cnc2[:], -C2)

            acc1 = sp.tile([128, DIM], dt.float32r)
            acc2 = sp.tile([128, DIM], dt.float32r)
            zeros_f = sp.tile([128, DIM], dt.float32)
            nc.vector.memset(zeros_f[:], 0.0)
            nc.vector.tensor_copy(acc1[:], zeros_f[:])
            nc.vector.tensor_copy(acc2[:], zeros_f[:])

            vf_dram = dram.tile([b_loc, DIM], dt.bfloat16)

            # ================= PHASE A =================
            # Software-pipelined emission: group g's routing/apply is emitted
            # after group g+1's L1 tiles so engine FIFOs let the next group's
            # PSUM evacuations run ahead of the serial routing chain.
            vf_tiles = {}
            usq_tiles = {}

            def emit_l1_tiles(g):
                g0, gsz = g_starts[g], groups[g]
                g_tiles = range(g0, g0 + gsz)
                usq_g = rp.tile([128, 4 * gsz], dt.float32, tag="usq", bufs=2, name=f"usq_{g}")
                usq_tiles[g] = usq_g
                for t in g_tiles:
                    x_sb = xp.tile([128, DIM], dt.float32, tag="x", name=f"x_{t}")
                    nc.sync.dma_start(x_sb[:], x_d[t * 128 : (t + 1) * 128, :])
                    # transpose x tile -> xT (fp32 PE transpose, 8 blocks, 2 psum banks)
                    xT = xtp.tile([128, 8, 128], dt.float32r, tag="xT", name=f"xT_{t}")
                    for bh in range(2):
                        xt_ps = ps.tile([128, 4, 128], dt.float32, tag="bank", name=f"xt_{t}_{bh}")
                        for k in range(4):
                            kk = bh * 4 + k
                            nc.tensor.transpose(xt_ps[:, k, :], x_sb[:, kk * 128 : (kk + 1) * 128], ident[:])
                        nc.vector.tensor_copy(xT[:, bh * 4 : (bh + 1) * 4, :], xt_ps[:])
                    # L1 matmul: u[b, :] (batch-major), fp32r; capsule norms fused on ACT
                    u16 = up.tile([128, DIM], dt.bfloat16, tag="u16", name=f"u16_{t}")
                    for dh in range(2):
                        u_ps = ps.tile([128, 512], dt.float32, tag="bank", name=f"u_{t}_{dh}")
                        for k in range(8):
                            nc.tensor.matmul(
                                u_ps[:],
                                xT[:, k, :],
                                wfT[:, k, dh * 512 : (dh + 1) * 512],
                                start=(k == 0), stop=(k == 7),
                            )
                        scr = scp.tile([128, 512], dt.float32, tag="scr", name=f"scr_{t}_{dh}")
                        for n in range(2):
                            nn_ = dh * 2 + n
                            nc.scalar.activation(
                                scr[:, n * DCAPS : (n + 1) * DCAPS],
                                u_ps[:, n * DCAPS : (n + 1) * DCAPS],
                                AF.Square,
                                accum_out=usq_g[:, 4 * (t - g0) + nn_ : 4 * (t - g0) + nn_ + 1],
                            )
                        nc.scalar.copy(u16[:, dh * 512 : (dh + 1) * 512], u_ps[:])
                    vf_tiles[t] = u16

            def emit_routing_apply(g):
                g0, gsz = g_starts[g], groups[g]
                g_tiles = range(g0, g0 + gsz)
                usq = usq_tiles.pop(g)[:]
                nt = gsz * 4

                def view3(ap):
                    return ap.rearrange("p (g n) -> p g n", n=4)

                sq = rp.tile([128, nt], dt.float32, tag="r_sq", name=f"sq_{g}")
                den = rp.tile([128, nt], dt.float32, tag="r_den", name=f"den_{g}")
                gg = rp.tile([128, nt], dt.float32, tag="r_g", name=f"gg_{g}")
                bb = rp.tile([128, nt], dt.float32, tag="r_b", name=f"bb_{g}")
                ee = rp.tile([128, nt], dt.float32, tag="r_e", name=f"ee_{g}")
                ss = rp.tile([128, gsz], dt.float32, tag="r_s", name=f"ss_{g}")
                cc = rp.tile([128, nt], dt.float32, tag="r_c", name=f"cc_{g}")
                uv = rp.tile([128, nt], dt.float32, tag="r_uv", name=f"uv_{g}")
                al = rp.tile([128, nt], dt.float32, tag="r_al", name=f"al_{g}")

                def emit_softmax(b_ap, shift, out_ap):
                    # out = softmax over capsule groups of 4 (shifted exp)
                    nc.scalar.activation(ee[:], b_ap, AF.Exp, bias=shift[:])
                    nc.vector.tensor_reduce(ss[:], view3(ee[:]), axis=mybir.AxisListType.X, op=ALU.add)
                    nc.vector.reciprocal(ss[:], ss[:])
                    nc.vector.tensor_tensor(
                        view3(out_ap), view3(ee[:]),
                        ss[:, :, None].to_broadcast((128, gsz, 4)), ALU.mult)

                # rt = sqrt(usq); with t = c^2*usq:
                #   uv    = rt * t/(1+t)
                #   alpha = c^2 * rt / (1+t)
                rt = rp.tile([128, nt], dt.float32, tag="r_rt", name=f"rt_{g}")
                c2t = rp.tile([128, nt], dt.float32, tag="r_c2", name=f"c2_{g}")
                nc.scalar.activation(rt[:], usq, AF.Sqrt, bias=ceps11[:])
                # iter 1 (c = 1/4 -> t = usq/16)
                nc.vector.tensor_scalar_mul(sq[:], usq, 1.0 / 16.0)
                nc.vector.tensor_scalar_add(den[:], sq[:], 1.0)
                nc.vector.reciprocal(den[:], den[:])
                nc.vector.tensor_tensor(gg[:], sq[:], den[:], ALU.mult)
                nc.vector.tensor_tensor(bb[:], gg[:], rt[:], ALU.mult)
                # iter 2
                emit_softmax(bb[:], cnc1, cc[:])
                nc.scalar.activation(c2t[:], cc[:], AF.Square)
                nc.vector.tensor_tensor(sq[:], c2t[:], usq, ALU.mult)
                nc.vector.tensor_scalar_add(den[:], sq[:], 1.0)
                nc.vector.reciprocal(den[:], den[:])
                nc.vector.tensor_tensor(gg[:], sq[:], den[:], ALU.mult)
                nc.vector.tensor_tensor(uv[:], gg[:], rt[:], ALU.mult)
                nc.vector.tensor_tensor(bb[:], bb[:], uv[:], ALU.add)
                # final coefficients
                emit_softmax(bb[:], cnc2, cc[:])
                nc.scalar.activation(c2t[:], cc[:], AF.Square)
                nc.vector.tensor_tensor(sq[:], c2t[:], usq, ALU.mult)
                nc.vector.tensor_scalar_add(den[:], sq[:], 1.0)
                nc.vector.reciprocal(den[:], den[:])
                nc.vector.tensor_tensor(gg[:], rt[:], den[:], ALU.mult)
                nc.vector.tensor_tensor(al[:], gg[:], c2t[:], ALU.mult)

                # ---- apply alpha (split DVE/ACT), accumulate BN stats, spill vf ----
                for t in g_tiles:
                    u16 = vf_tiles.pop(t)
                    tl = t - g0
                    for n in range(NCAPS):
                        nc.vector.tensor_scalar_mul(
                            u16[:, n * DCAPS : (n + 1) * DCAPS],
                            u16[:, n * DCAPS : (n + 1) * DCAPS],
                            al[:, tl * 4 + n : tl * 4 + n + 1])
                    nc.vector.tensor_tensor(acc1[:], acc1[:], u16[:], ALU.add)
                    scr2 = scp.tile([128, DIM], dt.float32, tag="scr2", name=f"scr2_{t}")
                    nc.scalar.activation(scr2[:], u16[:], AF.Square)
                    nc.gpsimd.tensor_tensor(acc2[:], acc2[:], scr2[:], ALU.add)
                    nc.sync.dma_start(vf_dram[t * 128 : (t + 1) * 128, :], u16[:])

            for g in range(n_groups):
                emit_l1_tiles(g)
                if g >= 1:
                    emit_routing_apply(g - 1)
            emit_routing_apply(n_groups - 1)

            # ================= BN STATS + ALLREDUCE =================
            s_sb = sp.tile([1, 2 * DIM], dt.float32)
            for h in range(4):
                acc_src = acc1 if h < 2 else acc2
                s_ps = ps.tile([1, 512], dt.float32, tag="bank", name=f"sfold_{h}")
                nc.tensor.matmul(
                    s_ps[:],
                    ones_r[:], acc_src[:, (h % 2) * 512 : (h % 2 + 1) * 512],
                    start=True, stop=True)
                nc.vector.tensor_copy(s_sb[:, h * 512 : (h + 1) * 512], s_ps[:])
            cc_in = dram.tile([1, 2 * DIM], dt.float32)
            cc_out = dram.tile([1, 2 * DIM], dt.float32, addr_space="Shared")
            nc.sync.dma_start(cc_in[:], s_sb[:])
            nc.gpsimd.collective_compute(
                "AllReduce", ALU.add,
                ins=[cc_in[:].opt()], outs=[cc_out[:].opt()],
                replica_groups=[list(range(NCORES))])
            # load global sums partition-scattered: Sg[p, o] = cc_out[o*128+p]
            Sg = sp.tile([128, 16], dt.float32)
            nc.sync.dma_start(Sg[:], cc_out[:].squeeze(0).rearrange("(o p) -> p o", p=128))
            # fold over capsules: dc = tt*128+p ; S1c[:, tt] = sum_n Sg[:, 2n+tt]
            S1c = sp.tile([128, 2], dt.float32)
            S2c = sp.tile([128, 2], dt.float32)
            for tt in range(2):
                nc.vector.tensor_reduce(
                    S1c[:, tt : tt + 1], Sg[:, tt : 8 : 2],
                    axis=mybir.AxisListType.X, op=ALU.add)
                nc.vector.tensor_reduce(
                    S2c[:, tt : tt + 1], Sg[:, 8 + tt : 16 : 2],
                    axis=mybir.AxisListType.X, op=ALU.add)
            mean = sp.tile([128, 2], dt.float32)
            var = sp.tile([128, 2], dt.float32)
            a2 = sp.tile([128, 2], dt.float32)
            m2 = sp.tile([128, 2], dt.float32)
            nc.vector.tensor_copy(mean[:], S1c[:])
            nc.vector.tensor_copy(var[:], S2c[:])
            tmp22 = sp.tile([128, 2], dt.float32)
            nc.vector.tensor_tensor(tmp22[:], mean[:], mean[:], ALU.mult)
            nc.vector.tensor_tensor(var[:], var[:], tmp22[:], ALU.subtract)
            nc.scalar.activation(var[:], var[:], AF.Sqrt, bias=cepsbn[:])
            nc.vector.reciprocal(var[:], var[:])          # rstd
            nc.vector.tensor_tensor(a2[:], gam2[:], var[:], ALU.mult)
            nc.vector.tensor_tensor(m2[:], mean[:], a2[:], ALU.mult)
            nc.vector.tensor_tensor(m2[:], bet2[:], m2[:], ALU.subtract)
            m2b = sp.tile([128, 2], dt.bfloat16)
            nc.vector.tensor_copy(m2b[:], m2[:])

            # b1eff[e] = b1[e] + sum_j W1[e, j] * m[j], computed as a row
            # [1, 1024] via wide matmuls, then partition-scattered via DRAM.
            b1m_row = sp.tile([1, DIM], dt.float32)
            for h in range(2):
                br_ps = ps.tile([1, 512], dt.float32, tag="bank", name=f"b1r_{h}")
                for k in range(8):
                    nc.tensor.matmul(
                        br_ps[:], m2b[:, k % 2 : k % 2 + 1],
                        w1T[:, k, h * 512 : (h + 1) * 512],
                        start=(k == 0), stop=(k == 7))
                nc.vector.tensor_copy(b1m_row[:, h * 512 : (h + 1) * 512], br_ps[:])
            b1m_dram = dram.tile([1, DIM], dt.float32)
            nc.sync.dma_start(b1m_dram[:], b1m_row[:])
            b1m_pe = sp.tile([128, 8], dt.float32)
            nc.sync.dma_start(b1m_pe[:], b1m_dram[:].squeeze(0).rearrange("(k p) -> p k", p=128))
            b1eff = sp.tile([128, 8], dt.float32)
            nc.vector.tensor_tensor(b1eff[:], b1pe[:], b1m_pe[:], ALU.add)

            # ================= PHASE B: MLP =================
            for c in range(n_chunks):
                vfT = vtp.tile([128, 8, chunk_rows], dt.bfloat16, tag="vfT")
                nc.sync.dma_start_transpose(
                    vfT[:], vf_dram[c * chunk_rows : (c + 1) * chunk_rows, :])
                # BN scale applied to activations (a is per-j = per-partition)
                for k in range(8):
                    nc.vector.tensor_scalar_mul(
                        vfT[:, k, :], vfT[:, k, :], a2[:, k % 2 : k % 2 + 1])
                hT = htp.tile([128, 8, chunk_rows], dt.bfloat16, tag="hT")
                for ec in range(8):
                    h_ps = ps.tile([128, chunk_rows], dt.float32, tag="bank", name=f"h_{c}_{ec}")
                    for k in range(8):
                        nc.tensor.matmul(
                            h_ps[:], w1T[:, k, ec * 128 : (ec + 1) * 128],
                            vfT[:, k, :], start=(k == 0), stop=(k == 7))
                    nc.scalar.activation(
                        hT[:, ec, :], h_ps[:], AF.Relu, bias=b1eff[:, ec : ec + 1])
                for bs in range(n_bsub):
                    o_sb = op_.tile([128, DIM], dt.float32, tag="osb")
                    for ot in range(2):
                        o_ps = ps.tile([128, 512], dt.float32, tag="bank", name=f"o_{c}_{bs}_{ot}")
                        for k in range(8):
                            nc.tensor.matmul(
                                o_ps[:], hT[:, k, bs * 128 : (bs + 1) * 128],
                                w2T[:, k, ot * 512 : (ot + 1) * 512],
                                start=(k == 0), stop=(k == 7))
                        nc.vector.tensor_tensor(
                            o_sb[:, ot * 512 : (ot + 1) * 512], o_ps[:],
                            b2bc[:, ot * 512 : (ot + 1) * 512], ALU.add)
                    r0 = c * chunk_rows + bs * 128
                    nc.gpsimd.dma_start(out_d[r0 : r0 + 128, :], o_sb[:])

    nc.compile()
    return nc


# effective total batch for BN statistics (mutable for small-scale testing)
B_TOTAL_EFF = [B_TOTAL]


def _prep_weights(W, gamma, beta, W1, b1, W2, b2):
    bf16 = ml_dtypes.bfloat16
    Wf = np.ascontiguousarray(W.reshape(DIM, DIM))           # [j, i]
    wfT = np.ascontiguousarray(Wf.T.reshape(8, 128, DIM).transpose(1, 0, 2))  # [p, k, j]
    w1T = np.ascontiguousarray(W1.T.reshape(8, 128, DIM).transpose(1, 0, 2)).astype(bf16)
    w2T = np.ascontiguousarray(W2.T.reshape(8, 128, DIM).transpose(1, 0, 2)).astype(bf16)
    b1pe = np.ascontiguousarray(b1.reshape(8, 128).T).astype(np.float32)
    b2row = np.ascontiguousarray(b2.reshape(1, DIM)).astype(np.float32)
    gam2 = np.ascontiguousarray(gamma.reshape(2, 128).T).astype(np.float32)
    bet2 = np.ascontiguousarray(beta.reshape(2, 128).T).astype(np.float32)
    return dict(wfT=wfT.astype(np.float32), w1T=w1T, w2T=w2T,
                b1pe=b1pe, b2row=b2row, gam2=gam2, bet2=bet2,
                ident=np.eye(128, dtype=np.float32))


def kernel(x, W, gamma, beta, W1, b1, W2, b2):
    x = np.asarray(x, np.float32)
    B = x.shape[0]
    b_loc = B // NCORES
    B_TOTAL_EFF[0] = B
    key = (b_loc, B)
    if key not in _CACHE:
        _CACHE[key] = build_kernel(b_loc)
    nc = _CACHE[key]
    wmaps = _prep_weights(np.asarray(W, np.float32), np.asarray(gamma, np.float32),
                          np.asarray(beta, np.float32), np.asarray(W1, np.float32),
                          np.asarray(b1, np.float32), np.asarray(W2, np.float32),
                          np.asarray(b2, np.float32))
    in_maps = [dict(wmaps, x=np.ascontiguousarray(x[i * b_loc : (i + 1) * b_loc]))
               for i in range(NCORES)]
    res = run_bass_kernel_spmd(nc, in_maps, core_ids=list(range(NCORES)))
    out = np.concatenate([res.results[i]["out"] for i in range(NCORES)], axis=0)
    return out
